# revision 6
# baseline (speedup 1.0000x reference)
"""Bahdanau-attention kernel for TRN2, data-parallel over 8 NeuronCores.

Math: the reference applies softmax over the LAST axis of scores, which has
size 1 — softmax over a singleton axis is identically 1.0 (exp(x-x)/exp(x-x)).
Therefore:
    attn_weights = ones(bs, sq, 21, 7, 1)
    attn_out     = attn_weights * keys = broadcast(keys, (bs, sq, 21, 7, 256))
independent of queries / masks / all projection weights. The kernel is a pure
DMA problem: per core, read its keys shard into SBUF and write it back 21x
(broadcast over the query axis), plus memset-ones for the weights output.

Per-core traffic: read 1.75 MiB + write 36.9 MiB ≈ 40.5 MB ⇒ ~113 us at the
~358 GB/s per-core HBM limit.
"""

import numpy as np

from concourse import bass, mybir
from concourse.bass_utils import run_bass_kernel_spmd

BS, SQ, NQ, NK, D = 16, 128, 21, 7, 256
N_CORES = 8
BPC = BS // N_CORES  # batches per core
ROW = NK * D  # contiguous floats per (b, s): 1792
W_ROW = NQ * NK  # attn_weights floats per (b, s): 147


def _build() -> bass.Bass:
    nc = bass.Bass()
    keys_in = nc.declare_dram_parameter(
        "keys", [BPC, SQ, ROW], mybir.dt.float32, isOutput=False
    )
    ones_in = nc.declare_dram_parameter(
        "ones", [SQ, W_ROW], mybir.dt.float32, isOutput=False
    )
    attn_out = nc.declare_dram_parameter(
        "attn_out", [BPC, SQ, NQ, ROW], mybir.dt.float32, isOutput=True
    )
    attn_w = nc.declare_dram_parameter(
        "attn_w", [BPC, SQ, W_ROW], mybir.dt.float32, isOutput=True
    )

    with (
        nc.Block() as block,
        nc.semaphore("in_sem") as in_sem,
        nc.semaphore("out_sem") as out_sem,
        nc.sbuf_tensor("kt", [SQ, BPC, ROW], mybir.dt.float32) as kt,
    ):

        @block.sync
        def _(sync: bass.BassEngine):
            # keys shard -> SBUF (128p, BPC, 1792), DRAM side iterated (s, b, d)
            sync.dma_start(out=kt[:], in_=keys_in[:].transpose([1, 0, 2])).then_inc(
                in_sem, 16
            )
            # weights: DRAM->DRAM ones broadcast over b, iterated (s, b, r)
            sync.dma_start(
                out=attn_w[:].transpose([1, 0, 2]),
                in_=ones_in[:].unsqueeze(1).broadcast_to((SQ, BPC, W_ROW)),
            ).then_inc(out_sem, 16)
            sync.wait_ge(in_sem, 16)
            # broadcast write per b: (128p, 21, 1792) from q-stride-0 source
            for b in range(BPC):
                sync.dma_start(
                    out=attn_out[b],
                    in_=kt[:, b].unsqueeze(1).broadcast_to((SQ, NQ, ROW)),
                ).then_inc(out_sem, 16)
            sync.wait_ge(out_sem, 48)

    return nc


_NC_CACHE: list = []


def kernel(**inputs: np.ndarray):
    keys = np.ascontiguousarray(
        np.asarray(inputs["keys"], dtype=np.float32).reshape(BS, SQ, ROW)
    )
    if not _NC_CACHE:
        _NC_CACHE.append(_build())
    nc = _NC_CACHE[0]

    ones = np.ones((SQ, W_ROW), dtype=np.float32)
    in_maps = [
        {"keys": keys[c * BPC : (c + 1) * BPC], "ones": ones} for c in range(N_CORES)
    ]
    res = run_bass_kernel_spmd(nc, in_maps, core_ids=list(range(N_CORES)))

    attn_out = np.concatenate(
        [r["attn_out"].reshape(BPC, SQ, NQ, NK, D) for r in res.results], axis=0
    )
    attn_w = np.concatenate(
        [r["attn_w"].reshape(BPC, SQ, NQ, NK, 1) for r in res.results], axis=0
    )
    return attn_out, attn_w
